# revision 4
# baseline (speedup 1.0000x reference)
"""TT-dense layer (BayesKerasDense): y = relu(x @ M + b), M given as a
4-core tensor-train. Strategy: the TT ranks (16) are large relative to the
mode sizes (8), so the TT sweep costs as many FLOPs as the dense matmul but
with 16x-larger intermediates and a full re-layout between stages. We
therefore materialize the dense M = TT(core0..core3) once on the host
(0.34 GMAC, trivial) and run a data-parallel dense matmul on 8 NeuronCores:
each core computes y_local[512, 4096] = relu(xT_local.T @ M + b) in bf16
with fp32 PSUM accumulation. The bias is folded into the accumulation as a
K=1 matmul (ones.T @ b); relu is fused into the PSUM->SBUF evacuation on
the scalar engine.
"""

import numpy as np
import ml_dtypes

import concourse.bacc as bacc
import concourse.mybir as mybir
import concourse.tile as tile
from concourse.bass_utils import run_bass_kernel_spmd

N_CORES = 8
B = 4096          # global batch
BL = B // N_CORES # per-core batch (512)
D = 4096          # n_in == n_out
BF16 = mybir.dt.bfloat16
F32 = mybir.dt.float32

N_TILES = D // 512    # 8 column tiles of 512
K_TILES = D // 128    # 32 contraction chunks of 128
M_TILES = BL // 128   # 4 batch tiles of 128


def _build_module():
    nc = bacc.Bacc("TRN2", target_bir_lowering=False, debug=False, num_devices=N_CORES)
    xt_d = nc.dram_tensor("xt", [D, BL], BF16, kind="ExternalInput")
    mw_d = nc.dram_tensor("mw", [D, D], BF16, kind="ExternalInput")
    b_d = nc.dram_tensor("bv", [D], BF16, kind="ExternalInput")
    y_d = nc.dram_tensor("y", [BL, D], F32, kind="ExternalOutput")

    with tile.TileContext(nc) as tc:
        with (
            tc.tile_pool(name="const", bufs=1) as cpool,
            tc.tile_pool(name="mpool", bufs=4) as mpool,
            tc.tile_pool(name="ypool", bufs=4) as ypool,
            tc.tile_pool(name="pspool", bufs=8, space="PSUM") as pspool,
        ):
            # x^T resident in SBUF: [128 partitions, K_TILES, BL] bf16
            xt_sb = cpool.tile([128, K_TILES, BL], BF16)
            for kc in range(K_TILES):
                nc.sync.dma_start(
                    out=xt_sb[:, kc, :], in_=xt_d[kc * 128 : (kc + 1) * 128, :]
                )
            b_sb = cpool.tile([1, D], BF16)
            nc.sync.dma_start(out=b_sb[:, :], in_=b_d[None, :])
            ones_sb = cpool.tile([1, 128], BF16)
            nc.gpsimd.memset(ones_sb[:], 1.0)

            for n in range(N_TILES):
                ns = slice(n * 512, (n + 1) * 512)
                ps = [
                    pspool.tile([128, 512], F32, name=f"ps_{n}_{m}", tag="ps")
                    for m in range(M_TILES)
                ]
                # bias via K=1 matmul: out[128,512] = ones[1,128].T @ b[1,512]
                for m in range(M_TILES):
                    nc.tensor.matmul(
                        ps[m][:], ones_sb[:, :], b_sb[:, ns], start=True, stop=False
                    )
                for k in range(K_TILES):
                    mt = mpool.tile([128, 512], BF16, name=f"mt_{n}_{k}", tag="mt")
                    nc.sync.dma_start(
                        out=mt[:], in_=mw_d[k * 128 : (k + 1) * 128, ns]
                    )
                    for m in range(M_TILES):
                        nc.tensor.matmul(
                            ps[m][:],
                            xt_sb[:, k, m * 128 : (m + 1) * 128],
                            mt[:],
                            start=False,
                            stop=(k == K_TILES - 1),
                        )
                for m in range(M_TILES):
                    yt = ypool.tile([128, 512], F32, name=f"yt_{n}_{m}", tag="yt")
                    nc.scalar.activation(
                        yt[:], ps[m][:], mybir.ActivationFunctionType.Relu
                    )
                    nc.sync.dma_start(
                        out=y_d[m * 128 : (m + 1) * 128, ns], in_=yt[:]
                    )
    nc.compile()
    return nc


def _materialize_dense(core0, core1, core2, core3) -> np.ndarray:
    """M[(a0,a1,a2,a3),(b0,b1,b2,b3)] from TT cores [r,a,b,q], row-major."""
    t = np.asarray(core0, np.float32).reshape(8, 8, 16)        # a0,b0,r1
    t = np.tensordot(t, np.asarray(core1, np.float32), axes=([2], [0]))
    # a0,b0,a1,b1,r2
    t = np.tensordot(t, np.asarray(core2, np.float32), axes=([4], [0]))
    # a0,b0,a1,b1,a2,b2,r3
    t = np.tensordot(t, np.asarray(core3, np.float32), axes=([6], [0]))[..., 0]
    # a0,b0,a1,b1,a2,b2,a3,b3
    return np.ascontiguousarray(
        t.transpose(0, 2, 4, 6, 1, 3, 5, 7).reshape(D, D)
    )


_module_cache: list = []


def kernel(x, core0, core1, core2, core3, b):
    bf = ml_dtypes.bfloat16
    M = _materialize_dense(core0, core1, core2, core3)
    Mb = M.astype(bf)
    bb = np.asarray(b, np.float32).astype(bf)
    x = np.asarray(x, np.float32)

    in_maps = []
    for c in range(N_CORES):
        xt = np.ascontiguousarray(x[c * BL : (c + 1) * BL].T).astype(bf)
        in_maps.append({"xt": xt, "mw": Mb, "bv": bb})

    if not _module_cache:
        _module_cache.append(_build_module())
    nc = _module_cache[0]
    res = run_bass_kernel_spmd(nc, in_maps, core_ids=list(range(N_CORES)))
    return np.concatenate([res.results[c]["y"] for c in range(N_CORES)], axis=0)


# revision 24
# speedup vs baseline: 1.1612x; 1.1612x over previous
"""TT-dense layer (BayesKerasDense): y = relu(x @ M + b), M given as a
4-core tensor-train. Strategy: the TT ranks (16) are large relative to the
mode sizes (8), so the TT sweep costs as many FLOPs as the dense matmul but
with 16x-larger intermediates and a full re-layout between stages. We
therefore materialize the dense M = TT(core0..core3) once on the host
(0.34 GMAC, trivial) and run a data-parallel dense matmul on 8 NeuronCores:
each core computes y_local[512, 4096] = relu(xT_local.T @ M + b) in bf16
with fp32 PSUM accumulation. The bias is folded into the accumulation as a
K=1 matmul (ones.T @ b); relu is fused into the PSUM->SBUF evacuation on
the scalar/vector engines.
"""

import numpy as np
import ml_dtypes

import concourse.bacc as bacc
import concourse.mybir as mybir
import concourse.tile as tile
from concourse.bass_utils import run_bass_kernel_spmd

N_CORES = 8
B = 4096          # global batch
BL = B // N_CORES # per-core batch (512)
D = 4096          # n_in == n_out
BF16 = mybir.dt.bfloat16
F32 = mybir.dt.float32

N_TILES = D // 512    # 8 column tiles of 512
K_TILES = D // 128    # 32 contraction chunks of 128
M_TILES = BL // 128   # 4 batch tiles of 128


def _build_module(
    mpool_bufs: int = 8,
    xt_mode: str = "swdge_each",
    split_last_n: bool = False,
    bias_mode: str = "evac",
    prefetch_mt: int = 0,
):
    nc = bacc.Bacc("TRN2", target_bir_lowering=False, debug=False, num_devices=N_CORES)
    xt_d = nc.dram_tensor("xt", [D, BL], BF16, kind="ExternalInput")
    mw_d = nc.dram_tensor("mw", [D, D], BF16, kind="ExternalInput")
    b_shape = [128, D] if bias_mode == "evac" else [D]
    b_d = nc.dram_tensor("bv", b_shape, BF16, kind="ExternalInput")
    y_d = nc.dram_tensor("y", [BL, D], F32, kind="ExternalOutput")

    with tile.TileContext(nc) as tc:
        with (
            tc.tile_pool(name="const", bufs=1) as cpool,
            tc.tile_pool(name="mpool", bufs=mpool_bufs) as mpool,
            tc.tile_pool(name="ypool", bufs=4) as ypool,
            tc.tile_pool(name="pspool", bufs=8, space="PSUM") as pspool,
        ):
            # x^T resident in SBUF: [128 partitions, K_TILES, BL] bf16.
            # Loads are interleaved with the n=0 M-tile stream so the first
            # matmuls aren't head-of-line blocked behind the whole 4MB.
            xt_sb = cpool.tile([128, K_TILES, BL], BF16)
            if bias_mode == "evac":
                # small single-row copy, loaded first: feeds the bias matmuls
                # of the first/last col-tiles
                b0_sb = cpool.tile([1, D], BF16)
                nc.sync.dma_start(out=b0_sb[:, :], in_=b_d[0:1, :])
                # full replicated bias for the evacuation adds; DMA emission
                # deferred until after the n=0 tile stream so it doesn't
                # head-of-line block the first matmuls' inputs.
                b_sb = cpool.tile([128, D], BF16)
            else:
                b_sb = cpool.tile([1, D], BF16)
                nc.sync.dma_start(out=b_sb[:, :], in_=b_d[None, :])
                b0_sb = b_sb
            ones_sb = cpool.tile([1, 128], BF16)
            nc.gpsimd.memset(ones_sb[:], 1.0)

            def load_xt(k):
                if xt_mode == "swdge_each":
                    # first chunk on HWDGE (fast first-byte) so the k=0
                    # matmuls start ASAP; the rest on SWDGE in parallel
                    eng = nc.sync if k == 0 else nc.gpsimd
                    eng.dma_start(
                        out=xt_sb[:, k, :], in_=xt_d[k * 128 : (k + 1) * 128, :]
                    )
                elif xt_mode == "split":
                    eng = nc.sync if k % 2 == 0 else nc.gpsimd
                    eng.dma_start(
                        out=xt_sb[:, k, :], in_=xt_d[k * 128 : (k + 1) * 128, :]
                    )
                elif xt_mode == "swdge_chunk4":
                    if k % 4 == 0:
                        src = xt_d.rearrange("(c p) b -> p c b", p=128)
                        nc.gpsimd.dma_start(
                            out=xt_sb[:, k : k + 4, :], in_=src[:, k : k + 4, :]
                        )
                else:
                    raise ValueError(xt_mode)

            # (n-tile index, column offset, column width)
            col_tiles = []
            for n in range(N_TILES):
                if split_last_n and n == N_TILES - 1:
                    col_tiles.append((n, n * 512, 384))
                    col_tiles.append((n, n * 512 + 384, 128))
                else:
                    col_tiles.append((n, n * 512, 512))

            for ci, (n, c0, cw) in enumerate(col_tiles):
                ns = slice(c0, c0 + cw)
                ps = [
                    pspool.tile([128, 512], F32, name=f"ps_{ci}_{m}", tag="ps")
                    for m in range(M_TILES)
                ]
                mts = {}
                if ci == 0 and prefetch_mt:
                    for k in range(prefetch_mt):
                        load_xt(k)
                        mt = mpool.tile(
                            [128, 512], BF16, name=f"mt_{ci}_{k}", tag="mt"
                        )
                        nc.sync.dma_start(
                            out=mt[:, :cw], in_=mw_d[k * 128 : (k + 1) * 128, ns]
                        )
                        mts[k] = mt
                # first col-tile: bias matmuls fill the initial DMA wait and
                # warm the PE clock; last col-tile: they make the tail
                # evacuation a single relu op instead of add+relu
                bias_by_matmul = bias_mode == "matmul" or (
                    bias_mode == "evac" and ci in (0, len(col_tiles) - 1)
                )
                if bias_by_matmul:
                    # out[128,cw] = ones[1,128].T @ b[1,cw]
                    for m in range(M_TILES):
                        nc.tensor.matmul(
                            ps[m][:, :cw], ones_sb[:, :], b0_sb[0:1, ns],
                            start=True, stop=False,
                        )
                for k in range(K_TILES):
                    if k in mts:
                        mt = mts[k]
                    else:
                        if ci == 0:
                            load_xt(k)
                        mt = mpool.tile(
                            [128, 512], BF16, name=f"mt_{ci}_{k}", tag="mt"
                        )
                        nc.sync.dma_start(
                            out=mt[:, :cw], in_=mw_d[k * 128 : (k + 1) * 128, ns]
                        )
                    for m in range(M_TILES):
                        nc.tensor.matmul(
                            ps[m][:, :cw],
                            xt_sb[:, k, m * 128 : (m + 1) * 128],
                            mt[:, :cw],
                            start=(not bias_by_matmul and k == 0),
                            stop=(k == K_TILES - 1),
                        )
                if ci == 0 and bias_mode == "evac":
                    # queued behind the n=0 M stream on HWDGE; needed by the
                    # first evacuation (~29us in)
                    nc.sync.dma_start(out=b_sb[:, :], in_=b_d[:, :])
                for m in range(M_TILES):
                    yt = ypool.tile([128, 512], F32, name=f"yt_{ci}_{m}", tag="yt")
                    if bias_mode == "evac" and not bias_by_matmul:
                        nc.vector.tensor_tensor(
                            yt[:, :cw], ps[m][:, :cw], b_sb[:, ns],
                            op=mybir.AluOpType.add,
                        )
                        nc.scalar.activation(
                            yt[:, :cw], yt[:, :cw],
                            mybir.ActivationFunctionType.Relu,
                        )
                    elif m % 2 == 0:
                        nc.scalar.activation(
                            yt[:, :cw], ps[m][:, :cw],
                            mybir.ActivationFunctionType.Relu,
                        )
                    else:
                        nc.vector.tensor_scalar_max(yt[:, :cw], ps[m][:, :cw], 0.0)
                    dma_eng = nc.sync if m % 2 == 0 else nc.gpsimd
                    dma_eng.dma_start(
                        out=y_d[m * 128 : (m + 1) * 128, ns], in_=yt[:, :cw]
                    )
    nc.compile()
    return nc


def _materialize_dense(core0, core1, core2, core3) -> np.ndarray:
    """M[(a0,a1,a2,a3),(b0,b1,b2,b3)] from TT cores [r,a,b,q], row-major."""
    t = np.asarray(core0, np.float32).reshape(8, 8, 16)        # a0,b0,r1
    t = np.tensordot(t, np.asarray(core1, np.float32), axes=([2], [0]))
    # a0,b0,a1,b1,r2
    t = np.tensordot(t, np.asarray(core2, np.float32), axes=([4], [0]))
    # a0,b0,a1,b1,a2,b2,r3
    t = np.tensordot(t, np.asarray(core3, np.float32), axes=([6], [0]))[..., 0]
    # a0,b0,a1,b1,a2,b2,a3,b3
    return np.ascontiguousarray(
        t.transpose(0, 2, 4, 6, 1, 3, 5, 7).reshape(D, D)
    )


_module_cache: list = []


def kernel(x, core0, core1, core2, core3, b):
    bf = ml_dtypes.bfloat16
    M = _materialize_dense(core0, core1, core2, core3)
    Mb = M.astype(bf)
    # bias replicated across the 128 PSUM partitions for the evacuation add
    bb = np.ascontiguousarray(
        np.broadcast_to(np.asarray(b, np.float32).astype(bf), (128, D))
    )
    x = np.asarray(x, np.float32)

    in_maps = []
    for c in range(N_CORES):
        xt = np.ascontiguousarray(x[c * BL : (c + 1) * BL].T).astype(bf)
        in_maps.append({"xt": xt, "mw": Mb, "bv": bb})

    if not _module_cache:
        _module_cache.append(_build_module())
    nc = _module_cache[0]
    res = run_bass_kernel_spmd(nc, in_maps, core_ids=list(range(N_CORES)))
    return np.concatenate([res.results[c]["y"] for c in range(N_CORES)], axis=0)


# revision 25
# speedup vs baseline: 1.1816x; 1.0175x over previous
"""TT-dense layer (BayesKerasDense): y = relu(x @ M + b), M given as a
4-core tensor-train. Strategy: the TT ranks (16) are large relative to the
mode sizes (8), so the TT sweep costs as many FLOPs as the dense matmul but
with 16x-larger intermediates and a full re-layout between stages. We
therefore materialize the dense M = TT(core0..core3) once on the host
(0.34 GMAC, trivial) and run a data-parallel dense matmul on 8 NeuronCores:
each core computes y_local[512, 4096] = relu(xT_local.T @ M + b) in bf16
with fp32 PSUM accumulation. The bias is folded into the accumulation as a
K=1 matmul (ones.T @ b); relu is fused into the PSUM->SBUF evacuation on
the scalar/vector engines.
"""

import sys

import numpy as np
import ml_dtypes

try:
    import concourse.bacc as bacc
except ImportError:  # fallback for environments without the site hook
    sys.path.insert(0, "/opt/trn_rl_repo")
    import concourse.bacc as bacc
import concourse.mybir as mybir
import concourse.tile as tile
from concourse.bass_utils import run_bass_kernel_spmd

N_CORES = 8
B = 4096          # global batch
BL = B // N_CORES # per-core batch (512)
D = 4096          # n_in == n_out
BF16 = mybir.dt.bfloat16
F32 = mybir.dt.float32

N_TILES = D // 512    # 8 column tiles of 512
K_TILES = D // 128    # 32 contraction chunks of 128
M_TILES = BL // 128   # 4 batch tiles of 128


def _build_module(
    mpool_bufs: int = 8,
    xt_mode: str = "swdge_each",
    split_last_n: bool = False,
    bias_mode: str = "evac",
    prefetch_mt: int = 0,
):
    nc = bacc.Bacc("TRN2", target_bir_lowering=False, debug=False, num_devices=N_CORES)
    xt_d = nc.dram_tensor("xt", [D, BL], BF16, kind="ExternalInput")
    mw_d = nc.dram_tensor("mw", [D, D], BF16, kind="ExternalInput")
    b_shape = [128, D] if bias_mode == "evac" else [D]
    b_d = nc.dram_tensor("bv", b_shape, BF16, kind="ExternalInput")
    y_d = nc.dram_tensor("y", [BL, D], F32, kind="ExternalOutput")

    with tile.TileContext(nc) as tc:
        with (
            tc.tile_pool(name="const", bufs=1) as cpool,
            tc.tile_pool(name="mpool", bufs=mpool_bufs) as mpool,
            tc.tile_pool(name="ypool", bufs=4) as ypool,
            tc.tile_pool(name="pspool", bufs=8, space="PSUM") as pspool,
        ):
            # x^T resident in SBUF: [128 partitions, K_TILES, BL] bf16.
            # Loads are interleaved with the n=0 M-tile stream so the first
            # matmuls aren't head-of-line blocked behind the whole 4MB.
            xt_sb = cpool.tile([128, K_TILES, BL], BF16)
            if bias_mode == "evac":
                # small single-row copy, loaded first: feeds the bias matmuls
                # of the first/last col-tiles
                b0_sb = cpool.tile([1, D], BF16)
                nc.sync.dma_start(out=b0_sb[:, :], in_=b_d[0:1, :])
                # full replicated bias for the evacuation adds; DMA emission
                # deferred until after the n=0 tile stream so it doesn't
                # head-of-line block the first matmuls' inputs.
                b_sb = cpool.tile([128, D], BF16)
            else:
                b_sb = cpool.tile([1, D], BF16)
                nc.sync.dma_start(out=b_sb[:, :], in_=b_d[None, :])
                b0_sb = b_sb
            ones_sb = cpool.tile([1, 128], BF16)
            nc.gpsimd.memset(ones_sb[:], 1.0)

            def load_xt(k):
                if xt_mode == "swdge_each":
                    # first chunk on HWDGE (fast first-byte) so the k=0
                    # matmuls start ASAP; the rest on SWDGE in parallel
                    eng = nc.sync if k == 0 else nc.gpsimd
                    eng.dma_start(
                        out=xt_sb[:, k, :], in_=xt_d[k * 128 : (k + 1) * 128, :]
                    )
                elif xt_mode == "split":
                    eng = nc.sync if k % 2 == 0 else nc.gpsimd
                    eng.dma_start(
                        out=xt_sb[:, k, :], in_=xt_d[k * 128 : (k + 1) * 128, :]
                    )
                elif xt_mode == "swdge_chunk4":
                    if k % 4 == 0:
                        src = xt_d.rearrange("(c p) b -> p c b", p=128)
                        nc.gpsimd.dma_start(
                            out=xt_sb[:, k : k + 4, :], in_=src[:, k : k + 4, :]
                        )
                else:
                    raise ValueError(xt_mode)

            # (n-tile index, column offset, column width)
            col_tiles = []
            for n in range(N_TILES):
                if split_last_n and n == N_TILES - 1:
                    col_tiles.append((n, n * 512, 384))
                    col_tiles.append((n, n * 512 + 384, 128))
                else:
                    col_tiles.append((n, n * 512, 512))

            for ci, (n, c0, cw) in enumerate(col_tiles):
                ns = slice(c0, c0 + cw)
                ps = [
                    pspool.tile([128, 512], F32, name=f"ps_{ci}_{m}", tag="ps")
                    for m in range(M_TILES)
                ]
                mts = {}
                if ci == 0 and prefetch_mt:
                    for k in range(prefetch_mt):
                        load_xt(k)
                        mt = mpool.tile(
                            [128, 512], BF16, name=f"mt_{ci}_{k}", tag="mt"
                        )
                        nc.sync.dma_start(
                            out=mt[:, :cw], in_=mw_d[k * 128 : (k + 1) * 128, ns]
                        )
                        mts[k] = mt
                # first col-tile: bias matmuls fill the initial DMA wait and
                # warm the PE clock; last col-tile: they make the tail
                # evacuation a single relu op instead of add+relu
                bias_by_matmul = bias_mode == "matmul" or (
                    bias_mode == "evac" and ci in (0, len(col_tiles) - 1)
                )
                if bias_by_matmul:
                    # out[128,cw] = ones[1,128].T @ b[1,cw]
                    for m in range(M_TILES):
                        nc.tensor.matmul(
                            ps[m][:, :cw], ones_sb[:, :], b0_sb[0:1, ns],
                            start=True, stop=False,
                        )
                for k in range(K_TILES):
                    if k in mts:
                        mt = mts[k]
                    else:
                        if ci == 0:
                            load_xt(k)
                        mt = mpool.tile(
                            [128, 512], BF16, name=f"mt_{ci}_{k}", tag="mt"
                        )
                        nc.sync.dma_start(
                            out=mt[:, :cw], in_=mw_d[k * 128 : (k + 1) * 128, ns]
                        )
                    for m in range(M_TILES):
                        nc.tensor.matmul(
                            ps[m][:, :cw],
                            xt_sb[:, k, m * 128 : (m + 1) * 128],
                            mt[:, :cw],
                            start=(not bias_by_matmul and k == 0),
                            stop=(k == K_TILES - 1),
                        )
                if ci == 0 and bias_mode == "evac":
                    # queued behind the n=0 M stream on HWDGE; needed by the
                    # first evacuation (~29us in)
                    nc.sync.dma_start(out=b_sb[:, :], in_=b_d[:, :])
                for m in range(M_TILES):
                    yt = ypool.tile([128, 512], F32, name=f"yt_{ci}_{m}", tag="yt")
                    if bias_mode == "evac" and not bias_by_matmul:
                        nc.vector.tensor_tensor(
                            yt[:, :cw], ps[m][:, :cw], b_sb[:, ns],
                            op=mybir.AluOpType.add,
                        )
                        nc.scalar.activation(
                            yt[:, :cw], yt[:, :cw],
                            mybir.ActivationFunctionType.Relu,
                        )
                    elif m % 2 == 0:
                        nc.scalar.activation(
                            yt[:, :cw], ps[m][:, :cw],
                            mybir.ActivationFunctionType.Relu,
                        )
                    else:
                        nc.vector.tensor_scalar_max(yt[:, :cw], ps[m][:, :cw], 0.0)
                    dma_eng = nc.sync if m % 2 == 0 else nc.gpsimd
                    dma_eng.dma_start(
                        out=y_d[m * 128 : (m + 1) * 128, ns], in_=yt[:, :cw]
                    )
    nc.compile()
    return nc


def _materialize_dense(core0, core1, core2, core3) -> np.ndarray:
    """M[(a0,a1,a2,a3),(b0,b1,b2,b3)] from TT cores [r,a,b,q], row-major."""
    t = np.asarray(core0, np.float32).reshape(8, 8, 16)        # a0,b0,r1
    t = np.tensordot(t, np.asarray(core1, np.float32), axes=([2], [0]))
    # a0,b0,a1,b1,r2
    t = np.tensordot(t, np.asarray(core2, np.float32), axes=([4], [0]))
    # a0,b0,a1,b1,a2,b2,r3
    t = np.tensordot(t, np.asarray(core3, np.float32), axes=([6], [0]))[..., 0]
    # a0,b0,a1,b1,a2,b2,a3,b3
    return np.ascontiguousarray(
        t.transpose(0, 2, 4, 6, 1, 3, 5, 7).reshape(D, D)
    )


_module_cache: list = []


def kernel(x, core0, core1, core2, core3, b):
    bf = ml_dtypes.bfloat16
    M = _materialize_dense(core0, core1, core2, core3)
    Mb = M.astype(bf)
    # bias replicated across the 128 PSUM partitions for the evacuation add
    bb = np.ascontiguousarray(
        np.broadcast_to(np.asarray(b, np.float32).astype(bf), (128, D))
    )
    x = np.asarray(x, np.float32)

    in_maps = []
    for c in range(N_CORES):
        xt = np.ascontiguousarray(x[c * BL : (c + 1) * BL].T).astype(bf)
        in_maps.append({"xt": xt, "mw": Mb, "bv": bb})

    if not _module_cache:
        _module_cache.append(_build_module())
    nc = _module_cache[0]
    res = run_bass_kernel_spmd(nc, in_maps, core_ids=list(range(N_CORES)))
    return np.concatenate([res.results[c]["y"] for c in range(N_CORES)], axis=0)


# revision 27
# speedup vs baseline: 1.1828x; 1.0011x over previous
"""TT-dense layer (BayesKerasDense): y = relu(x @ M + b), M given as a
4-core tensor-train. Strategy: the TT ranks (16) are large relative to the
mode sizes (8), so the TT sweep costs as many FLOPs as the dense matmul but
with 16x-larger intermediates and a full re-layout between stages. We
therefore materialize the dense M = TT(core0..core3) once on the host
(0.34 GMAC, trivial) and run a data-parallel dense matmul on 8 NeuronCores:
each core computes y_local[512, 4096] = relu(xT_local.T @ M + b) in bf16
with fp32 PSUM accumulation. The bias is folded into the accumulation as a
K=1 matmul (ones.T @ b); relu is fused into the PSUM->SBUF evacuation on
the scalar/vector engines.
"""

import sys

import numpy as np
import ml_dtypes

try:
    import concourse.bacc as bacc
except ImportError:  # fallback for environments without the site hook
    sys.path.insert(0, "/opt/trn_rl_repo")
    import concourse.bacc as bacc
import concourse.mybir as mybir
import concourse.tile as tile
from concourse.bass_utils import run_bass_kernel_spmd

N_CORES = 8
B = 4096          # global batch
BL = B // N_CORES # per-core batch (512)
D = 4096          # n_in == n_out
BF16 = mybir.dt.bfloat16
F32 = mybir.dt.float32

N_TILES = D // 512    # 8 column tiles of 512
K_TILES = D // 128    # 32 contraction chunks of 128
M_TILES = BL // 128   # 4 batch tiles of 128


def _build_module(
    mpool_bufs: int = 8,
    xt_mode: str = "swdge_each",
    split_last_n: bool = False,
    bias_mode: str = "evac",
    prefetch_mt: int = 0,
):
    nc = bacc.Bacc("TRN2", target_bir_lowering=False, debug=False, num_devices=N_CORES)
    xt_d = nc.dram_tensor("xt", [D, BL], BF16, kind="ExternalInput")
    mw_d = nc.dram_tensor("mw", [D, D], BF16, kind="ExternalInput")
    b_shape = [128, D] if bias_mode == "evac" else [D]
    b_d = nc.dram_tensor("bv", b_shape, BF16, kind="ExternalInput")
    y_d = nc.dram_tensor("y", [BL, D], F32, kind="ExternalOutput")

    with tile.TileContext(nc) as tc:
        with (
            tc.tile_pool(name="const", bufs=1) as cpool,
            tc.tile_pool(name="mpool", bufs=mpool_bufs) as mpool,
            tc.tile_pool(name="ypool", bufs=4) as ypool,
            tc.tile_pool(name="pspool", bufs=8, space="PSUM") as pspool,
        ):
            # x^T resident in SBUF: [128 partitions, K_TILES, BL] bf16.
            # Loads are interleaved with the n=0 M-tile stream so the first
            # matmuls aren't head-of-line blocked behind the whole 4MB.
            xt_sb = cpool.tile([128, K_TILES, BL], BF16)
            if bias_mode == "evac":
                # small single-row copy, loaded first: feeds the bias matmuls
                # of the first/last col-tiles
                b0_sb = cpool.tile([1, D], BF16)
                nc.sync.dma_start(out=b0_sb[:, :], in_=b_d[0:1, :])
                # full replicated bias for the evacuation adds; DMA emission
                # deferred until after the n=0 tile stream so it doesn't
                # head-of-line block the first matmuls' inputs.
                b_sb = cpool.tile([128, D], BF16)
            else:
                b_sb = cpool.tile([1, D], BF16)
                nc.sync.dma_start(out=b_sb[:, :], in_=b_d[None, :])
                b0_sb = b_sb
            ones_sb = cpool.tile([1, 128], BF16)
            nc.gpsimd.memset(ones_sb[:], 1.0)

            def load_xt(k):
                if xt_mode == "swdge_each":
                    # first chunk on HWDGE (fast first-byte) so the k=0
                    # matmuls start ASAP; the rest on SWDGE in parallel
                    eng = nc.sync if k == 0 else nc.gpsimd
                    eng.dma_start(
                        out=xt_sb[:, k, :], in_=xt_d[k * 128 : (k + 1) * 128, :]
                    )
                elif xt_mode == "split":
                    eng = nc.sync if k % 2 == 0 else nc.gpsimd
                    eng.dma_start(
                        out=xt_sb[:, k, :], in_=xt_d[k * 128 : (k + 1) * 128, :]
                    )
                elif xt_mode == "swdge_chunk4":
                    if k % 4 == 0:
                        src = xt_d.rearrange("(c p) b -> p c b", p=128)
                        nc.gpsimd.dma_start(
                            out=xt_sb[:, k : k + 4, :], in_=src[:, k : k + 4, :]
                        )
                else:
                    raise ValueError(xt_mode)

            # (n-tile index, column offset, column width)
            col_tiles = []
            for n in range(N_TILES):
                if split_last_n and n == N_TILES - 1:
                    col_tiles.append((n, n * 512, 384))
                    col_tiles.append((n, n * 512 + 384, 128))
                else:
                    col_tiles.append((n, n * 512, 512))

            for ci, (n, c0, cw) in enumerate(col_tiles):
                ns = slice(c0, c0 + cw)
                ps = [
                    pspool.tile([128, 512], F32, name=f"ps_{ci}_{m}", tag="ps")
                    for m in range(M_TILES)
                ]
                mts = {}
                if ci == 0 and prefetch_mt:
                    for k in range(prefetch_mt):
                        load_xt(k)
                        mt = mpool.tile(
                            [128, 512], BF16, name=f"mt_{ci}_{k}", tag="mt"
                        )
                        nc.sync.dma_start(
                            out=mt[:, :cw], in_=mw_d[k * 128 : (k + 1) * 128, ns]
                        )
                        mts[k] = mt
                # first col-tile: bias matmuls fill the initial DMA wait and
                # warm the PE clock; last col-tile: they make the tail
                # evacuation a single relu op instead of add+relu
                bias_by_matmul = bias_mode == "matmul" or (
                    bias_mode == "evac" and ci in (0, len(col_tiles) - 1)
                )
                if bias_by_matmul:
                    # out[128,cw] = ones[1,128].T @ b[1,cw]
                    for m in range(M_TILES):
                        nc.tensor.matmul(
                            ps[m][:, :cw], ones_sb[:, :], b0_sb[0:1, ns],
                            start=True, stop=False,
                        )
                for k in range(K_TILES):
                    if k in mts:
                        mt = mts[k]
                    else:
                        if ci == 0:
                            load_xt(k)
                        mt = mpool.tile(
                            [128, 512], BF16, name=f"mt_{ci}_{k}", tag="mt"
                        )
                        nc.sync.dma_start(
                            out=mt[:, :cw], in_=mw_d[k * 128 : (k + 1) * 128, ns]
                        )
                    for m in range(M_TILES):
                        nc.tensor.matmul(
                            ps[m][:, :cw],
                            xt_sb[:, k, m * 128 : (m + 1) * 128],
                            mt[:, :cw],
                            start=(not bias_by_matmul and k == 0),
                            stop=(k == K_TILES - 1),
                        )
                if ci == 0 and bias_mode == "evac":
                    # queued behind the n=0 M stream on HWDGE; needed by the
                    # first evacuation (~29us in)
                    nc.sync.dma_start(out=b_sb[:, :], in_=b_d[:, :])
                for m in range(M_TILES):
                    yt = ypool.tile([128, 512], F32, name=f"yt_{ci}_{m}", tag="yt")
                    if bias_mode == "evac" and not bias_by_matmul:
                        nc.vector.tensor_tensor(
                            yt[:, :cw], ps[m][:, :cw], b_sb[:, ns],
                            op=mybir.AluOpType.add,
                        )
                        nc.scalar.activation(
                            yt[:, :cw], yt[:, :cw],
                            mybir.ActivationFunctionType.Relu,
                        )
                    elif m % 2 == 0:
                        nc.scalar.activation(
                            yt[:, :cw], ps[m][:, :cw],
                            mybir.ActivationFunctionType.Relu,
                        )
                    else:
                        nc.vector.tensor_scalar_max(yt[:, :cw], ps[m][:, :cw], 0.0)
                    if ci == len(col_tiles) - 1:
                        # tail stores: four distinct launch queues so the HW
                        # DMA engines drain them in parallel
                        dma_eng = (nc.sync, nc.gpsimd, nc.scalar, nc.sync)[m]
                    else:
                        dma_eng = nc.sync if m % 2 == 0 else nc.gpsimd
                    dma_eng.dma_start(
                        out=y_d[m * 128 : (m + 1) * 128, ns], in_=yt[:, :cw]
                    )
    nc.compile()
    return nc


def _materialize_dense(core0, core1, core2, core3) -> np.ndarray:
    """M[(a0,a1,a2,a3),(b0,b1,b2,b3)] from TT cores [r,a,b,q], row-major."""
    t = np.asarray(core0, np.float32).reshape(8, 8, 16)        # a0,b0,r1
    t = np.tensordot(t, np.asarray(core1, np.float32), axes=([2], [0]))
    # a0,b0,a1,b1,r2
    t = np.tensordot(t, np.asarray(core2, np.float32), axes=([4], [0]))
    # a0,b0,a1,b1,a2,b2,r3
    t = np.tensordot(t, np.asarray(core3, np.float32), axes=([6], [0]))[..., 0]
    # a0,b0,a1,b1,a2,b2,a3,b3
    return np.ascontiguousarray(
        t.transpose(0, 2, 4, 6, 1, 3, 5, 7).reshape(D, D)
    )


_module_cache: list = []


def kernel(x, core0, core1, core2, core3, b):
    bf = ml_dtypes.bfloat16
    M = _materialize_dense(core0, core1, core2, core3)
    Mb = M.astype(bf)
    # bias replicated across the 128 PSUM partitions for the evacuation add
    bb = np.ascontiguousarray(
        np.broadcast_to(np.asarray(b, np.float32).astype(bf), (128, D))
    )
    x = np.asarray(x, np.float32)

    in_maps = []
    for c in range(N_CORES):
        xt = np.ascontiguousarray(x[c * BL : (c + 1) * BL].T).astype(bf)
        in_maps.append({"xt": xt, "mw": Mb, "bv": bb})

    if not _module_cache:
        _module_cache.append(_build_module())
    nc = _module_cache[0]
    res = run_bass_kernel_spmd(nc, in_maps, core_ids=list(range(N_CORES)))
    return np.concatenate([res.results[c]["y"] for c in range(N_CORES)], axis=0)
